# revision 56
# baseline (speedup 1.0000x reference)
"""Multi-head attention (softmax+1) for TRN2, 8 NeuronCores.

Sharding: data-parallel over batch B=2 (4 cores per batch) x tensor-parallel
over the 16 heads (4 heads per core).  Each core computes its 4 heads'
QKV projections, attention, and a partial output projection; the host sums
the 4 partials per batch and adds the output bias.

Single software-pipelined loop over 128 (qq, p, ktile) entries, paced by the
scalar engine's exp stream (the roofline: 128 x [128,1024] ACTIVATEs):
  scores^T[k,q] (f16 PE, head-paired row groups) -> exp on ACT written
  DIRECTLY AS fp8e4 into DoubleRow-paired k-planes -> U^T accumulated with
  fp8 DoubleRow matmuls (2x PE rate; V' split into fp8 hi+lo to keep V
  precision ~f16).  QKV projections, the output projection, and the
  normalization (1/(1+den) via ones-column denominators) are interleaved
  into the PE/DVE/GPSIMD slack under the exp stream.
"""

import sys

if "/opt/trn_rl_repo" not in sys.path:
    sys.path.insert(0, "/opt/trn_rl_repo")

import numpy as np

import concourse.bass as bass
import concourse.mybir as mybir
import concourse.tile as tile
from concourse import bacc
from concourse.bass_utils import run_bass_kernel_spmd

F32 = mybir.dt.float32
F16 = mybir.dt.float16
F8 = mybir.dt.float8e4
EXP = mybir.ActivationFunctionType.Exp
DR = mybir.MatmulPerfMode.DoubleRow
SUB = mybir.AluOpType.subtract
MULT = mybir.AluOpType.mult

B, S, DM = 2, 2048, 1024
H, HD = 16, 64
SCALE = HD ** -0.5
HLOC = 4              # heads per core
CD = HLOC * HD        # 256 local head dims
VW = HD + 1           # 65: V columns + ones column per head
W260 = HLOC * VW      # 260
MC = DM // 128        # 8 contraction chunks for projections
NKT = S // 128        # 16 k tiles
NPAIR = NKT // 2      # 8 ktile pairs (fp8 DoubleRow planes)
VP = 80               # fp8 V' plane stride (pad 65 -> 80, 16-aligned)

_CACHE = {}
LAST_RESULT = None


def _build():
    nc = bacc.Bacc()
    dp = nc.declare_dram_parameter
    # x inputs pre-tiled on host into contiguous 512-col blocks:
    # blob[b][d 128][m 8][512] so each block DMA reads one contiguous 1MB
    xq_d = dp("xq", [4 * 128, MC * 512], F16, isOutput=False)
    xk_d = dp("xk", [4 * 128, MC * 512], F16, isOutput=False)
    xv_d = dp("xv", [4 * 128, MC * 512], F16, isOutput=False)
    # weights pre-shuffled on host to [128][chunk][cols] blob order so each
    # loads with a single dma_start
    wq_d = dp("wq", [128, MC * CD], F16, isOutput=False)   # (SCALE*Wq)^T blob
    wk_d = dp("wk", [128, MC * CD], F16, isOutput=False)
    wv_d = dp("wv", [128, MC * W260], F16, isOutput=False)
    wo_d = dp("wo", [128, 2 * DM], F16, isOutput=False)
    bq_d = dp("bq", [128, 2], F32, isOutput=False)   # bias cols per 128-pair
    bk_d = dp("bk", [128, 2], F32, isOutput=False)
    bv_d = dp("bv", [1, W260], F16, isOutput=False)  # [bv_h | 1.0] blocks
    on_d = dp("ones1", [1, 128], F16, isOutput=False)
    out_d = dp("out", [S, DM], F16, isOutput=True)   # partial (pre-bo) projection
    scr_d = dp("scratch", [1, 8], F32, isOutput=True)  # keeps warm-up mms alive

    with tile.TileContext(nc) as tc:
        with tc.tile_pool(name="weights", bufs=1) as wpool, \
             tc.tile_pool(name="persist", bufs=1) as perst, \
             tc.tile_pool(name="xres", bufs=1) as xres:
            wq_sb = wpool.tile([128, MC, CD], F16)
            wk_sb = wpool.tile([128, MC, CD], F16)
            wv_sb = wpool.tile([128, MC, W260], F16)
            wo_sb = wpool.tile([128, 2, DM], F16)
            bq_sb = wpool.tile([128, 2], F32)
            bk_sb = wpool.tile([128, 2], F32)
            bv_sb = wpool.tile([1, W260], F16)
            on_sb = wpool.tile([1, 128], F16)

            xq_sb = xres.tile([128, MC, S], F16)
            xk_sb = xres.tile([128, MC, S], F16)
            xv_sb = xres.tile([128, MC, S], F16)

            qt_sb = perst.tile([128, 2, S], F16)   # [d(2 heads), pair, q]
            kt_sb = perst.tile([128, 2, S], F16)
            at_sb = perst.tile([128, 2, S], F16)   # normalized attn out^T
            v8hi = perst.tile([128, NPAIR, HLOC, 2, VP], F8)
            v8lo = perst.tile([128, NPAIR, HLOC, 2, VP], F8)

            # ---- prologue DMAs, split across both HWDGE queues (SP + ACT),
            # one contiguous 1MB blob per 512-col block, deadline order ----
            def dma_x(eng, x_sb, x_d, b):
                eng.dma_start(out=x_sb[:, :, b * 512:(b + 1) * 512],
                              in_=x_d.ap()[b * 128:(b + 1) * 128, :])

            # SP queue: Q-proj + K-proj critical path, first-needed-first.
            # Early transfers split in halves so more DMA engines engage
            # during the queue ramp.
            nc.sync.dma_start(out=wq_sb[0:64], in_=wq_d.ap()[0:64, :])
            nc.sync.dma_start(out=wq_sb[64:128], in_=wq_d.ap()[64:128, :])
            nc.sync.dma_start(out=bq_sb[:], in_=bq_d.ap())
            nc.sync.dma_start(out=wk_sb[0:64], in_=wk_d.ap()[0:64, :])
            nc.sync.dma_start(out=wk_sb[64:128], in_=wk_d.ap()[64:128, :])
            nc.sync.dma_start(out=bk_sb[:], in_=bk_d.ap())
            nc.sync.dma_start(out=xk_sb[0:64, :, 0:512], in_=xk_d.ap()[0:64, :])
            nc.sync.dma_start(out=xk_sb[64:128, :, 0:512], in_=xk_d.ap()[64:128, :])
            for b in range(1, 4):
                dma_x(nc.sync, xk_sb, xk_d, b)       # K proj jb1..3
            for b in range(1, 4):
                dma_x(nc.sync, xq_sb, xq_d, b)       # Q proj qq1..3

            # ACT queue: Q-proj qq0 input + V-proj critical path + wo
            nc.scalar.dma_start(out=xq_sb[0:64, :, 0:512],
                                in_=xq_d.ap()[0:64, :])
            nc.scalar.dma_start(out=xq_sb[64:128, :, 0:512],
                                in_=xq_d.ap()[64:128, :])
            nc.scalar.dma_start(out=bv_sb[:], in_=bv_d.ap())
            nc.scalar.dma_start(out=on_sb[:], in_=on_d.ap())
            nc.scalar.dma_start(out=wv_sb[:], in_=wv_d.ap())
            for b in range(4):
                dma_x(nc.scalar, xv_sb, xv_d, b)     # V proj kt 4b..4b+3
            nc.scalar.dma_start(out=wo_sb[:], in_=wo_d.ap())

            with tc.tile_pool(name="psc", bufs=2, space="PSUM") as psc, \
                 tc.tile_pool(name="put", bufs=2, space="PSUM") as put, \
                 tc.tile_pool(name="paux", bufs=2, space="PSUM") as paux, \
                 tc.tile_pool(name="ex8p", bufs=4) as ex8p, \
                 tc.tile_pool(name="obuf", bufs=2) as obuf, \
                 tc.tile_pool(name="npool", bufs=2) as npool:

                open_tiles = {}

                def qproj(qq, p, step=None):
                    # step None: all 8 mm; else 2 mm per step (0..3), spread
                    # across entries to stay under the ACT pacing budget
                    steps = range(4) if step is None else (step,)
                    for s in steps:
                        if s == 0:
                            open_tiles["q"] = paux.tile([128, 512], F32,
                                                        tag="aux", name=f"q{qq}{p}")
                        t = open_tiles["q"]
                        for m in (2 * s, 2 * s + 1):
                            nc.tensor.matmul(
                                t[:], wq_sb[:, m, p * 128:(p + 1) * 128],
                                xq_sb[:, m, qq * 512:(qq + 1) * 512],
                                start=(m == 0), stop=(m == MC - 1))
                        if s == 3:
                            nc.vector.tensor_scalar_add(
                                qt_sb[:, p, qq * 512:(qq + 1) * 512],
                                t[:], bq_sb[:, p:p + 1])

                def kproj(jb, p):
                    t = paux.tile([128, 512], F32, tag="aux", name=f"k{jb}{p}")
                    for m in range(MC):
                        nc.tensor.matmul(
                            t[:], wk_sb[:, m, p * 128:(p + 1) * 128],
                            xk_sb[:, m, jb * 512:(jb + 1) * 512],
                            start=(m == 0), stop=(m == MC - 1))
                    nc.vector.tensor_scalar_add(
                        kt_sb[:, p, jb * 512:(jb + 1) * 512], t[:], bk_sb[:, p:p + 1])

                def vproj(kt):
                    t = paux.tile([128, 512], F32, tag="aux", name=f"v{kt}")
                    nc.tensor.matmul(t[:, 0:W260], on_sb[:], bv_sb[:],
                                     start=True, stop=False)
                    for m in range(MC):
                        nc.tensor.matmul(
                            t[:, 0:W260],
                            xv_sb[:, m, kt * 128:(kt + 1) * 128],
                            wv_sb[:, m, :],
                            start=False, stop=(m == MC - 1))
                    pr, pl = kt // 2, kt % 2
                    # hi = fp8(v'), lo = fp8(v' - hi); 4 heads in one strided op
                    nc.vector.tensor_copy(
                        v8hi[:, pr, :, pl, 0:VW], t[:, 0:W260])
                    nc.vector.tensor_tensor(
                        out=v8lo[:, pr, :, pl, 0:VW], in0=t[:, 0:W260],
                        in1=v8hi[:, pr, :, pl, 0:VW], op=SUB)

                def outproj(ot, half=None, act_copy=False):
                    # half 0: cc=0 matmuls (start); half 1: cc=1 (stop) +
                    # copies + store.  half None: both.  act_copy: offload
                    # one PSUM->SBUF copy to the (idle) scalar engine.
                    halves = range(2) if half is None else (half,)
                    for cc in halves:
                        if cc == 0:
                            open_tiles["o"] = [
                                paux.tile([128, 512], F32, tag="aux",
                                          name=f"o{ot}{n}") for n in range(2)]
                        os_ = open_tiles["o"]
                        for n in range(2):
                            nc.tensor.matmul(
                                os_[n][:],
                                at_sb[:, cc, ot * 128:(ot + 1) * 128],
                                wo_sb[:, cc, n * 512:(n + 1) * 512],
                                start=(cc == 0), stop=(cc == 1))
                        if cc == 1:
                            ob = obuf.tile([128, 1024], F16, tag="ob", name="ob")
                            nc.vector.tensor_copy(ob[:, 0:512], os_[0][:])
                            if act_copy:
                                nc.scalar.copy(ob[:, 512:1024], os_[1][:])
                            else:
                                nc.vector.tensor_copy(ob[:, 512:1024], os_[1][:])
                            nc.sync.dma_start(
                                out=out_d.ap()[ot * 128:(ot + 1) * 128, :],
                                in_=ob[:])

                def normalize(qq, p, uts):
                    rs = []
                    for hh in range(2):
                        den = npool.tile([1, 512], F32, tag="den", name=f"d{hh}")
                        nc.vector.tensor_scalar_add(den[:], uts[hh][64:65, :], 1.0)
                        r = npool.tile([1, 512], F32, tag="r", name=f"r{hh}")
                        nc.vector.reciprocal_approx_fast(r[:], den[:])
                        rs.append(r)
                    rbs = []
                    for hh in range(2):
                        rb = npool.tile([64, 512], F32, tag="rb", name=f"b{hh}")
                        nc.gpsimd.partition_broadcast(rb[:], rs[hh][:])
                        rbs.append(rb)
                    for hh in range(2):
                        nc.vector.tensor_tensor(
                            out=at_sb[64 * hh:64 * hh + 64, p, qq * 512:(qq + 1) * 512],
                            in0=uts[hh][0:64, :], in1=rbs[hh][:], op=MULT)

                # ---- prologue compute: bare minimum before entry (0,0,0);
                # everything else rides the crunch-quarter filler slots ----
                qproj(0, 0)
                kproj(0, 0)

                # ---- main pipeline ----
                def u_accum(uts, p, j, exj, hh):
                    h = 2 * p + hh
                    nc.tensor.matmul(
                        uts[hh][:], v8hi[:, j, h, :, 0:VW], exj[:, :, hh, :],
                        start=(j == 0), stop=False, perf_mode=DR)
                    nc.tensor.matmul(
                        uts[hh][:], v8lo[:, j, h, :, 0:VW], exj[:, :, hh, :],
                        start=False, stop=(j == NPAIR - 1), perf_mode=DR)

                carry = None   # (uts, p, qq, exj) of the previous quarter
                for qq in range(4):
                    for p in range(2):
                        uts = (put.tile([65, 512], F32, tag="ut", name="ut0"),
                               put.tile([65, 512], F32, tag="ut", name="ut1"))
                        exi = None
                        expairs = {}
                        for i in range(NKT):
                            j = i // 2
                            sc = psc.tile([128, 1024], F32, tag="sc", name="sc")
                            for hh in range(2):
                                nc.tensor.matmul(
                                    sc[:, hh * 512:(hh + 1) * 512],
                                    kt_sb[64 * hh:64 * hh + 64, p, i * 128:(i + 1) * 128],
                                    qt_sb[64 * hh:64 * hh + 64, p, qq * 512:(qq + 1) * 512],
                                    start=True, stop=True)
                            if i % 2 == 0:
                                exi = ex8p.tile([128, 2, 2, 512], F8, tag="ex",
                                                name="ex")
                                expairs[j] = exi
                            nc.scalar.activation(out=exi[:, i % 2, :, :],
                                                 in_=sc[:], func=EXP)
                            # ---- interleaved fillers ----
                            if qq == 0 and p == 0:
                                if i == 0:
                                    qproj(0, 1)
                                else:
                                    vproj(i - 1)
                                    if i == 15:
                                        vproj(15)
                                if i in (2, 6, 10):
                                    kproj(i // 4 + 1, 0)
                                if i in (4, 8, 12, 14):
                                    kproj((i - 4) // 4 if i < 14 else 3, 1)
                            else:
                                if p == 1 and qq < 3 and 1 <= i <= 4:
                                    qproj(qq + 1, 0, step=i - 1)
                                if p == 1 and qq < 3 and 7 <= i <= 10:
                                    qproj(qq + 1, 1, step=i - 7)
                                if p == 0 and qq > 0 and 7 <= i <= 14:
                                    ot = (qq - 1) * 4 + (i - 7) // 2
                                    outproj(ot, half=(i - 7) % 2)
                            # ---- previous quarter's last U halves + normalize,
                            # carried into entries 1-3 so they never
                            # head-of-line block this quarter's scores ----
                            if 1 <= i <= 3 and carry is not None:
                                cu, cp, cqq, cex = carry
                                if i == 1:
                                    u_accum(cu, cp, NPAIR - 2, cex[NPAIR - 2], hh=1)
                                elif i == 2:
                                    u_accum(cu, cp, NPAIR - 1, cex[NPAIR - 1], hh=0)
                                else:
                                    u_accum(cu, cp, NPAIR - 1, cex[NPAIR - 1], hh=1)
                                    normalize(cqq, cp, cu)
                                    carry = None
                            # ---- U accumulation, one head per entry, lag 2 ----
                            if i >= 3 and i % 2 == 1:
                                u_accum(uts, p, (i - 3) // 2,
                                        expairs[(i - 3) // 2], hh=0)
                            if i >= 4 and i % 2 == 0:
                                jd = (i - 4) // 2
                                u_accum(uts, p, jd, expairs[jd], hh=1)
                        carry = (uts, p, qq, expairs)
                        expairs = {}
                # ---- epilogue: final quarter's U + normalize + output tiles ----
                cu, cp, cqq, cex = carry
                u_accum(cu, cp, NPAIR - 2, cex[NPAIR - 2], hh=1)
                u_accum(cu, cp, NPAIR - 1, cex[NPAIR - 1], hh=0)
                u_accum(cu, cp, NPAIR - 1, cex[NPAIR - 1], hh=1)
                normalize(cqq, cp, cu)
                # keep the PE warm (HAM 8/8) while the normalize chain runs.
                # Each mm READS the final exp pair so the scheduler cannot
                # hoist them earlier; the scratch store keeps them unpruned.
                wps = paux.tile([128, 512], F32, tag="aux", name="warm")
                wex = cex[NPAIR - 1]
                for w in range(20):
                    nc.tensor.matmul(wps[:], wex[:, 0, 0, 0:128],
                                     wex[:, w % 2, 0, :],
                                     start=(w == 0), stop=(w == 19))
                scr = obuf.tile([1, 8], F32, tag="scr", name="scr")
                nc.vector.tensor_copy(scr[:], wps[0:1, 0:8])
                nc.sync.dma_start(out=scr_d.ap(), in_=scr[:])
                for ot in range(12, 16):
                    outproj(ot, act_copy=True)

    nc.finalize()
    return nc


def kernel(query, key, value, Wq, bq, Wk, bk, Wv, bv, Wo, bo):
    global LAST_RESULT
    if "nc" not in _CACHE:
        _CACHE["nc"] = _build()
    nc = _CACHE["nc"]

    query = np.asarray(query, np.float32)
    key = np.asarray(key, np.float32)
    value = np.asarray(value, np.float32)
    Wq = np.asarray(Wq, np.float32)
    Wk = np.asarray(Wk, np.float32)
    Wv = np.asarray(Wv, np.float32)
    Wo = np.asarray(Wo, np.float32)
    bq = np.asarray(bq, np.float32)
    bk = np.asarray(bk, np.float32)
    bv = np.asarray(bv, np.float32)
    bo = np.asarray(bo, np.float32)

    def blob(x):
        # [S, DM] -> x^T [DM, S] -> [4*128, MC*512]: block b rows hold
        # [d 128][m 8][512] contiguously for single-descriptor-chain DMAs
        xT = x.T.astype(np.float16)                       # [DM, S]
        t = xT.reshape(MC, 128, 4, 512).transpose(2, 1, 0, 3)
        return np.ascontiguousarray(t.reshape(4 * 128, MC * 512))

    xqT = [blob(query[b]) for b in range(B)]
    xkT = [blob(key[b]) for b in range(B)]
    xvT = [blob(value[b]) for b in range(B)]

    def wblob(wT, nch):
        # [nch*128, C] -> [128, nch*C] matching SBUF [128][chunk][C] tiles
        C = wT.shape[1]
        t = wT.reshape(nch, 128, C).transpose(1, 0, 2)
        return np.ascontiguousarray(t.reshape(128, nch * C)).astype(np.float16)

    ones1 = np.ones((1, 128), np.float16)
    in_maps = []
    for c in range(8):
        b, hg = c // 4, c % 4
        r0 = hg * CD
        wq_s = wblob((Wq[r0:r0 + CD, :] * SCALE).T, MC)
        wk_s = wblob(Wk[r0:r0 + CD, :].T, MC)
        wo_s = wblob(Wo[:, r0:r0 + CD].T, 2)
        bq_s = np.ascontiguousarray((bq[r0:r0 + CD] * SCALE).reshape(2, 128).T)
        bk_s = np.ascontiguousarray(bk[r0:r0 + CD].reshape(2, 128).T)
        # V weights/bias in 260-layout: [64 cols of head | bias-1 col] x4
        wv260 = np.zeros((DM, W260), np.float32)
        bv260 = np.zeros((1, W260), np.float32)
        for hh in range(HLOC):
            wv260[:, hh * VW:hh * VW + HD] = Wv[r0 + hh * HD:r0 + (hh + 1) * HD, :].T
            bv260[0, hh * VW:hh * VW + HD] = bv[r0 + hh * HD:r0 + (hh + 1) * HD]
            bv260[0, hh * VW + HD] = 1.0
        in_maps.append({
            "xq": xqT[b], "xk": xkT[b], "xv": xvT[b],
            "wq": wq_s, "wk": wk_s, "wv": wblob(wv260, MC),
            "wo": wo_s, "bq": bq_s, "bk": bk_s, "bv": bv260.astype(np.float16),
            "ones1": ones1,
        })

    res = run_bass_kernel_spmd(nc, in_maps, core_ids=list(range(8)))
    LAST_RESULT = res

    out = np.empty((B, S, DM), np.float32)
    for b in range(B):
        acc = np.zeros((S, DM), np.float64)
        for hg in range(4):
            acc += res.results[b * 4 + hg]["out"]
        out[b] = (acc + bo.astype(np.float64)).astype(np.float32)
    return out


# revision 58
# speedup vs baseline: 1.0451x; 1.0451x over previous
"""Multi-head attention (softmax+1) for TRN2, 8 NeuronCores.

Sharding: data-parallel over batch B=2 (4 cores per batch) x tensor-parallel
over the 16 heads (4 heads per core).  Each core computes its 4 heads'
QKV projections, attention, and a partial output projection; the host sums
the 4 partials per batch and adds the output bias.

Single software-pipelined loop over 128 (qq, p, ktile) entries, paced by the
scalar engine's exp stream (the roofline: 128 x [128,1024] ACTIVATEs):
  scores^T[k,q] (f16 PE, head-paired row groups) -> exp on ACT written
  DIRECTLY AS fp8e4 into DoubleRow-paired k-planes -> U^T accumulated with
  fp8 DoubleRow matmuls (2x PE rate; V' split into fp8 hi+lo to keep V
  precision ~f16).  QKV projections, the output projection, and the
  normalization (1/(1+den) via ones-column denominators) are interleaved
  into the PE/DVE/GPSIMD slack under the exp stream.
"""

import sys

if "/opt/trn_rl_repo" not in sys.path:
    sys.path.insert(0, "/opt/trn_rl_repo")

import numpy as np

import concourse.bass as bass
import concourse.mybir as mybir
import concourse.tile as tile
from concourse import bacc
from concourse.bass_utils import run_bass_kernel_spmd

F32 = mybir.dt.float32
F16 = mybir.dt.float16
F8 = mybir.dt.float8e4
EXP = mybir.ActivationFunctionType.Exp
DR = mybir.MatmulPerfMode.DoubleRow
SUB = mybir.AluOpType.subtract
MULT = mybir.AluOpType.mult

B, S, DM = 2, 2048, 1024
H, HD = 16, 64
SCALE = HD ** -0.5
HLOC = 4              # heads per core
CD = HLOC * HD        # 256 local head dims
VW = HD + 1           # 65: V columns + ones column per head
W260 = HLOC * VW      # 260
MC = DM // 128        # 8 contraction chunks for projections
NKT = S // 128        # 16 k tiles
NPAIR = NKT // 2      # 8 ktile pairs (fp8 DoubleRow planes)
VP = 80               # fp8 V' plane stride (pad 65 -> 80, 16-aligned)

_CACHE = {}
LAST_RESULT = None


def _build():
    nc = bacc.Bacc()
    dp = nc.declare_dram_parameter
    # x inputs pre-tiled on host into contiguous 512-col blocks:
    # blob[b][d 128][m 8][512] so each block DMA reads one contiguous 1MB
    xq_d = dp("xq", [4 * 128, MC * 512], F16, isOutput=False)
    xk_d = dp("xk", [4 * 128, MC * 512], F16, isOutput=False)
    xv_d = dp("xv", [4 * 128, MC * 512], F16, isOutput=False)
    # weights pre-shuffled on host to [128][chunk][cols] blob order so each
    # loads with a single dma_start
    wq_d = dp("wq", [128, MC * CD], F16, isOutput=False)   # (SCALE*Wq)^T blob
    wk_d = dp("wk", [128, MC * CD], F16, isOutput=False)
    wv_d = dp("wv", [128, MC * W260], F16, isOutput=False)
    wo_d = dp("wo", [128, 2 * DM], F16, isOutput=False)
    bq_d = dp("bq", [128, 2], F32, isOutput=False)   # bias cols per 128-pair
    bk_d = dp("bk", [128, 2], F32, isOutput=False)
    bv_d = dp("bv", [1, W260], F16, isOutput=False)  # [bv_h | 1.0] blocks
    on_d = dp("ones1", [1, 128], F16, isOutput=False)
    out_d = dp("out", [S, DM], F16, isOutput=True)   # partial (pre-bo) projection
    scr_d = dp("scratch", [1, 8], F32, isOutput=True)  # keeps warm-up mms alive

    with tile.TileContext(nc) as tc:
        with tc.tile_pool(name="weights", bufs=1) as wpool, \
             tc.tile_pool(name="persist", bufs=1) as perst, \
             tc.tile_pool(name="xres", bufs=1) as xres:
            wq_sb = wpool.tile([128, MC, CD], F16)
            wk_sb = wpool.tile([128, MC, CD], F16)
            wv_sb = wpool.tile([128, MC, W260], F16)
            wo_sb = wpool.tile([128, 2, DM], F16)
            bq_sb = wpool.tile([128, 2], F32)
            bk_sb = wpool.tile([128, 2], F32)
            bv_sb = wpool.tile([1, W260], F16)
            on_sb = wpool.tile([1, 128], F16)

            xq_sb = xres.tile([128, MC, S], F16)
            xk_sb = xres.tile([128, MC, S], F16)
            xv_sb = xres.tile([128, MC, S], F16)

            qt_sb = perst.tile([128, 2, S], F16)   # [d(2 heads), pair, q]
            kt_sb = perst.tile([128, 2, S], F16)
            at_sb = perst.tile([128, 2, S], F16)   # normalized attn out^T
            v8hi = perst.tile([128, NPAIR, HLOC, 2, VP], F8)
            v8lo = perst.tile([128, NPAIR, HLOC, 2, VP], F8)

            # ---- prologue DMAs, split across both HWDGE queues (SP + ACT),
            # one contiguous 1MB blob per 512-col block, deadline order ----
            def dma_x(eng, x_sb, x_d, b):
                eng.dma_start(out=x_sb[:, :, b * 512:(b + 1) * 512],
                              in_=x_d.ap()[b * 128:(b + 1) * 128, :])

            # SP queue: Q-proj + K-proj critical path, first-needed-first
            nc.sync.dma_start(out=wq_sb[:], in_=wq_d.ap())
            nc.sync.dma_start(out=bq_sb[:], in_=bq_d.ap())
            nc.sync.dma_start(out=wk_sb[:], in_=wk_d.ap())
            nc.sync.dma_start(out=bk_sb[:], in_=bk_d.ap())
            for b in range(4):
                dma_x(nc.sync, xk_sb, xk_d, b)       # K proj jb0..3
            for b in range(1, 4):
                dma_x(nc.sync, xq_sb, xq_d, b)       # Q proj qq1..3

            # ACT queue: Q-proj qq0 input + V-proj critical path + wo
            nc.scalar.dma_start(out=xq_sb[:, 0:4, 0:512],
                                in_=xq_d.ap()[0:128, 0:2048])
            nc.scalar.dma_start(out=xq_sb[:, 4:8, 0:512],
                                in_=xq_d.ap()[0:128, 2048:4096])
            nc.scalar.dma_start(out=bv_sb[:], in_=bv_d.ap())
            nc.scalar.dma_start(out=on_sb[:], in_=on_d.ap())
            nc.scalar.dma_start(out=wv_sb[:], in_=wv_d.ap())
            for b in range(4):
                dma_x(nc.scalar, xv_sb, xv_d, b)     # V proj kt 4b..4b+3
            nc.scalar.dma_start(out=wo_sb[:], in_=wo_d.ap())

            with tc.tile_pool(name="psc", bufs=2, space="PSUM") as psc, \
                 tc.tile_pool(name="put", bufs=2, space="PSUM") as put, \
                 tc.tile_pool(name="paux", bufs=2, space="PSUM") as paux, \
                 tc.tile_pool(name="ex8p", bufs=4) as ex8p, \
                 tc.tile_pool(name="obuf", bufs=2) as obuf, \
                 tc.tile_pool(name="npool", bufs=2) as npool:

                open_tiles = {}

                def qproj(qq, p, step=None):
                    # step None: all 8 mm; else 2 mm per step (0..3), spread
                    # across entries to stay under the ACT pacing budget
                    steps = range(4) if step is None else (step,)
                    for s in steps:
                        if s == 0:
                            open_tiles["q"] = paux.tile([128, 512], F32,
                                                        tag="aux", name=f"q{qq}{p}")
                        t = open_tiles["q"]
                        for m in (2 * s, 2 * s + 1):
                            nc.tensor.matmul(
                                t[:], wq_sb[:, m, p * 128:(p + 1) * 128],
                                xq_sb[:, m, qq * 512:(qq + 1) * 512],
                                start=(m == 0), stop=(m == MC - 1))
                        if s == 3:
                            nc.vector.tensor_scalar_add(
                                qt_sb[:, p, qq * 512:(qq + 1) * 512],
                                t[:], bq_sb[:, p:p + 1])

                def kproj(jb, p):
                    t = paux.tile([128, 512], F32, tag="aux", name=f"k{jb}{p}")
                    for m in range(MC):
                        nc.tensor.matmul(
                            t[:], wk_sb[:, m, p * 128:(p + 1) * 128],
                            xk_sb[:, m, jb * 512:(jb + 1) * 512],
                            start=(m == 0), stop=(m == MC - 1))
                    nc.vector.tensor_scalar_add(
                        kt_sb[:, p, jb * 512:(jb + 1) * 512], t[:], bk_sb[:, p:p + 1])

                def vproj(kt):
                    t = paux.tile([128, 512], F32, tag="aux", name=f"v{kt}")
                    nc.tensor.matmul(t[:, 0:W260], on_sb[:], bv_sb[:],
                                     start=True, stop=False)
                    for m in range(MC):
                        nc.tensor.matmul(
                            t[:, 0:W260],
                            xv_sb[:, m, kt * 128:(kt + 1) * 128],
                            wv_sb[:, m, :],
                            start=False, stop=(m == MC - 1))
                    pr, pl = kt // 2, kt % 2
                    # hi = fp8(v'), lo = fp8(v' - hi); 4 heads in one strided op
                    nc.vector.tensor_copy(
                        v8hi[:, pr, :, pl, 0:VW], t[:, 0:W260])
                    nc.vector.tensor_tensor(
                        out=v8lo[:, pr, :, pl, 0:VW], in0=t[:, 0:W260],
                        in1=v8hi[:, pr, :, pl, 0:VW], op=SUB)

                def outproj(ot, half=None, act_copy=False):
                    # half 0: cc=0 matmuls (start); half 1: cc=1 (stop) +
                    # copies + store.  half None: both.  act_copy: offload
                    # one PSUM->SBUF copy to the (idle) scalar engine.
                    halves = range(2) if half is None else (half,)
                    for cc in halves:
                        if cc == 0:
                            open_tiles["o"] = [
                                paux.tile([128, 512], F32, tag="aux",
                                          name=f"o{ot}{n}") for n in range(2)]
                        os_ = open_tiles["o"]
                        for n in range(2):
                            nc.tensor.matmul(
                                os_[n][:],
                                at_sb[:, cc, ot * 128:(ot + 1) * 128],
                                wo_sb[:, cc, n * 512:(n + 1) * 512],
                                start=(cc == 0), stop=(cc == 1))
                        if cc == 1:
                            ob = obuf.tile([128, 1024], F16, tag="ob", name="ob")
                            nc.vector.tensor_copy(ob[:, 0:512], os_[0][:])
                            if act_copy:
                                nc.scalar.copy(ob[:, 512:1024], os_[1][:])
                            else:
                                nc.vector.tensor_copy(ob[:, 512:1024], os_[1][:])
                            nc.sync.dma_start(
                                out=out_d.ap()[ot * 128:(ot + 1) * 128, :],
                                in_=ob[:])

                def normalize(qq, p, uts):
                    rs = []
                    for hh in range(2):
                        den = npool.tile([1, 512], F32, tag="den", name=f"d{hh}")
                        nc.vector.tensor_scalar_add(den[:], uts[hh][64:65, :], 1.0)
                        r = npool.tile([1, 512], F32, tag="r", name=f"r{hh}")
                        nc.vector.reciprocal_approx_fast(r[:], den[:])
                        rs.append(r)
                    rbs = []
                    for hh in range(2):
                        rb = npool.tile([64, 512], F32, tag="rb", name=f"b{hh}")
                        nc.gpsimd.partition_broadcast(rb[:], rs[hh][:])
                        rbs.append(rb)
                    for hh in range(2):
                        nc.vector.tensor_tensor(
                            out=at_sb[64 * hh:64 * hh + 64, p, qq * 512:(qq + 1) * 512],
                            in0=uts[hh][0:64, :], in1=rbs[hh][:], op=MULT)

                # ---- prologue compute: bare minimum before entry (0,0,0);
                # everything else rides the crunch-quarter filler slots ----
                qproj(0, 0)
                kproj(0, 0)

                # ---- main pipeline ----
                def u_accum(uts, p, j, exj, hh):
                    h = 2 * p + hh
                    nc.tensor.matmul(
                        uts[hh][:], v8hi[:, j, h, :, 0:VW], exj[:, :, hh, :],
                        start=(j == 0), stop=False, perf_mode=DR)
                    nc.tensor.matmul(
                        uts[hh][:], v8lo[:, j, h, :, 0:VW], exj[:, :, hh, :],
                        start=False, stop=(j == NPAIR - 1), perf_mode=DR)

                carry = None   # (uts, p, qq, exj) of the previous quarter
                for qq in range(4):
                    for p in range(2):
                        uts = (put.tile([65, 512], F32, tag="ut", name="ut0"),
                               put.tile([65, 512], F32, tag="ut", name="ut1"))
                        exi = None
                        expairs = {}
                        for i in range(NKT):
                            j = i // 2
                            sc = psc.tile([128, 1024], F32, tag="sc", name="sc")
                            for hh in range(2):
                                nc.tensor.matmul(
                                    sc[:, hh * 512:(hh + 1) * 512],
                                    kt_sb[64 * hh:64 * hh + 64, p, i * 128:(i + 1) * 128],
                                    qt_sb[64 * hh:64 * hh + 64, p, qq * 512:(qq + 1) * 512],
                                    start=True, stop=True)
                            if i % 2 == 0:
                                exi = ex8p.tile([128, 2, 2, 512], F8, tag="ex",
                                                name="ex")
                                expairs[j] = exi
                            nc.scalar.activation(out=exi[:, i % 2, :, :],
                                                 in_=sc[:], func=EXP)
                            # ---- interleaved fillers ----
                            if qq == 0 and p == 0:
                                if i == 0:
                                    qproj(0, 1)
                                else:
                                    vproj(i - 1)
                                    if i == 15:
                                        vproj(15)
                                if i in (2, 6, 10):
                                    kproj(i // 4 + 1, 0)
                                if i in (4, 8, 12, 14):
                                    kproj((i - 4) // 4 if i < 14 else 3, 1)
                            else:
                                if p == 1 and qq < 3 and 1 <= i <= 4:
                                    qproj(qq + 1, 0, step=i - 1)
                                if p == 1 and qq < 3 and 7 <= i <= 10:
                                    qproj(qq + 1, 1, step=i - 7)
                                if p == 0 and qq > 0 and 7 <= i <= 14:
                                    ot = (qq - 1) * 4 + (i - 7) // 2
                                    outproj(ot, half=(i - 7) % 2)
                            # ---- previous quarter's last U halves + normalize,
                            # carried into entries 1-3 so they never
                            # head-of-line block this quarter's scores ----
                            if 1 <= i <= 3 and carry is not None:
                                cu, cp, cqq, cex = carry
                                if i == 1:
                                    u_accum(cu, cp, NPAIR - 2, cex[NPAIR - 2], hh=1)
                                elif i == 2:
                                    u_accum(cu, cp, NPAIR - 1, cex[NPAIR - 1], hh=0)
                                else:
                                    u_accum(cu, cp, NPAIR - 1, cex[NPAIR - 1], hh=1)
                                    normalize(cqq, cp, cu)
                                    carry = None
                            # ---- U accumulation, one head per entry, lag 2 ----
                            if i >= 3 and i % 2 == 1:
                                u_accum(uts, p, (i - 3) // 2,
                                        expairs[(i - 3) // 2], hh=0)
                            if i >= 4 and i % 2 == 0:
                                jd = (i - 4) // 2
                                u_accum(uts, p, jd, expairs[jd], hh=1)
                        carry = (uts, p, qq, expairs)
                        expairs = {}
                # ---- epilogue: final quarter's U + normalize + output tiles ----
                cu, cp, cqq, cex = carry
                u_accum(cu, cp, NPAIR - 2, cex[NPAIR - 2], hh=1)
                u_accum(cu, cp, NPAIR - 1, cex[NPAIR - 1], hh=0)
                u_accum(cu, cp, NPAIR - 1, cex[NPAIR - 1], hh=1)
                normalize(cqq, cp, cu)
                # keep the PE warm (HAM 8/8) while the normalize chain runs.
                # Each mm READS the final exp pair so the scheduler cannot
                # hoist them earlier; the scratch store keeps them unpruned.
                wps = paux.tile([128, 512], F32, tag="aux", name="warm")
                wex = cex[NPAIR - 1]
                for w in range(20):
                    nc.tensor.matmul(wps[:], wex[:, 0, 0, 0:128],
                                     wex[:, w % 2, 0, :],
                                     start=(w == 0), stop=(w == 19))
                scr = obuf.tile([1, 8], F32, tag="scr", name="scr")
                nc.vector.tensor_copy(scr[:], wps[0:1, 0:8])
                nc.sync.dma_start(out=scr_d.ap(), in_=scr[:])
                for ot in range(12, 16):
                    outproj(ot, act_copy=True)

    nc.finalize()
    return nc


def kernel(query, key, value, Wq, bq, Wk, bk, Wv, bv, Wo, bo):
    global LAST_RESULT
    if "nc" not in _CACHE:
        _CACHE["nc"] = _build()
    nc = _CACHE["nc"]

    query = np.asarray(query, np.float32)
    key = np.asarray(key, np.float32)
    value = np.asarray(value, np.float32)
    Wq = np.asarray(Wq, np.float32)
    Wk = np.asarray(Wk, np.float32)
    Wv = np.asarray(Wv, np.float32)
    Wo = np.asarray(Wo, np.float32)
    bq = np.asarray(bq, np.float32)
    bk = np.asarray(bk, np.float32)
    bv = np.asarray(bv, np.float32)
    bo = np.asarray(bo, np.float32)

    def blob(x):
        # [S, DM] -> x^T [DM, S] -> [4*128, MC*512]: block b rows hold
        # [d 128][m 8][512] contiguously for single-descriptor-chain DMAs
        xT = x.T.astype(np.float16)                       # [DM, S]
        t = xT.reshape(MC, 128, 4, 512).transpose(2, 1, 0, 3)
        return np.ascontiguousarray(t.reshape(4 * 128, MC * 512))

    xqT = [blob(query[b]) for b in range(B)]
    xkT = [blob(key[b]) for b in range(B)]
    xvT = [blob(value[b]) for b in range(B)]

    def wblob(wT, nch):
        # [nch*128, C] -> [128, nch*C] matching SBUF [128][chunk][C] tiles
        C = wT.shape[1]
        t = wT.reshape(nch, 128, C).transpose(1, 0, 2)
        return np.ascontiguousarray(t.reshape(128, nch * C)).astype(np.float16)

    ones1 = np.ones((1, 128), np.float16)
    in_maps = []
    for c in range(8):
        b, hg = c // 4, c % 4
        r0 = hg * CD
        wq_s = wblob((Wq[r0:r0 + CD, :] * SCALE).T, MC)
        wk_s = wblob(Wk[r0:r0 + CD, :].T, MC)
        wo_s = wblob(Wo[:, r0:r0 + CD].T, 2)
        bq_s = np.ascontiguousarray((bq[r0:r0 + CD] * SCALE).reshape(2, 128).T)
        bk_s = np.ascontiguousarray(bk[r0:r0 + CD].reshape(2, 128).T)
        # V weights/bias in 260-layout: [64 cols of head | bias-1 col] x4
        wv260 = np.zeros((DM, W260), np.float32)
        bv260 = np.zeros((1, W260), np.float32)
        for hh in range(HLOC):
            wv260[:, hh * VW:hh * VW + HD] = Wv[r0 + hh * HD:r0 + (hh + 1) * HD, :].T
            bv260[0, hh * VW:hh * VW + HD] = bv[r0 + hh * HD:r0 + (hh + 1) * HD]
            bv260[0, hh * VW + HD] = 1.0
        in_maps.append({
            "xq": xqT[b], "xk": xkT[b], "xv": xvT[b],
            "wq": wq_s, "wk": wk_s, "wv": wblob(wv260, MC),
            "wo": wo_s, "bq": bq_s, "bk": bk_s, "bv": bv260.astype(np.float16),
            "ones1": ones1,
        })

    res = run_bass_kernel_spmd(nc, in_maps, core_ids=list(range(8)))
    LAST_RESULT = res

    out = np.empty((B, S, DM), np.float32)
    for b in range(B):
        acc = np.zeros((S, DM), np.float64)
        for hg in range(4):
            acc += res.results[b * 4 + hg]["out"]
        out[b] = (acc + bo.astype(np.float64)).astype(np.float32)
    return out


# revision 59
# speedup vs baseline: 1.2367x; 1.1833x over previous
"""Multi-head attention (softmax+1) for TRN2, 8 NeuronCores.

Sharding: data-parallel over batch B=2 (4 cores per batch) x tensor-parallel
over the 16 heads (4 heads per core).  Each core computes its 4 heads'
QKV projections, attention, and a partial output projection; the host sums
the 4 partials per batch and adds the output bias.

Single software-pipelined loop over 128 (qq, p, ktile) entries, paced by the
scalar engine's exp stream (the roofline: 128 x [128,1024] ACTIVATEs):
  scores^T[k,q] (f16 PE, head-paired row groups) -> exp on ACT written
  DIRECTLY AS fp8e4 into DoubleRow-paired k-planes -> U^T accumulated with
  fp8 DoubleRow matmuls (2x PE rate; V' split into fp8 hi+lo to keep V
  precision ~f16).  QKV projections, the output projection, and the
  normalization (1/(1+den) via ones-column denominators) are interleaved
  into the PE/DVE/GPSIMD slack under the exp stream.
"""

import sys

if "/opt/trn_rl_repo" not in sys.path:
    sys.path.insert(0, "/opt/trn_rl_repo")

import numpy as np

import concourse.bass as bass
import concourse.mybir as mybir
import concourse.tile as tile
from concourse import bacc
from concourse.bass_utils import run_bass_kernel_spmd

F32 = mybir.dt.float32
F16 = mybir.dt.float16
F8 = mybir.dt.float8e4
EXP = mybir.ActivationFunctionType.Exp
DR = mybir.MatmulPerfMode.DoubleRow
SUB = mybir.AluOpType.subtract
MULT = mybir.AluOpType.mult

B, S, DM = 2, 2048, 1024
H, HD = 16, 64
SCALE = HD ** -0.5
HLOC = 4              # heads per core
CD = HLOC * HD        # 256 local head dims
VW = HD + 1           # 65: V columns + ones column per head
W260 = HLOC * VW      # 260
MC = DM // 128        # 8 contraction chunks for projections
NKT = S // 128        # 16 k tiles
NPAIR = NKT // 2      # 8 ktile pairs (fp8 DoubleRow planes)
VP = 80               # fp8 V' plane stride (pad 65 -> 80, 16-aligned)

_CACHE = {}
LAST_RESULT = None


def _build():
    nc = bacc.Bacc()
    dp = nc.declare_dram_parameter
    # x inputs pre-tiled on host into contiguous 512-col blocks:
    # blob[b][d 128][m 8][512] so each block DMA reads one contiguous 1MB
    xq_d = dp("xq", [4 * 128, MC * 512], F16, isOutput=False)
    xk_d = dp("xk", [4 * 128, MC * 512], F16, isOutput=False)
    xv_d = dp("xv", [4 * 128, MC * 512], F16, isOutput=False)
    # weights pre-shuffled on host to [128][chunk][cols] blob order so each
    # loads with a single dma_start
    wq_d = dp("wq", [128, MC * CD], F16, isOutput=False)   # (SCALE*Wq)^T blob
    wk_d = dp("wk", [128, MC * CD], F16, isOutput=False)
    wv_d = dp("wv", [128, MC * W260], F16, isOutput=False)
    wo_d = dp("wo", [128, 2 * DM], F16, isOutput=False)
    bq_d = dp("bq", [128, 2], F32, isOutput=False)   # bias cols per 128-pair
    bk_d = dp("bk", [128, 2], F32, isOutput=False)
    bv_d = dp("bv", [1, W260], F16, isOutput=False)  # [bv_h | 1.0] blocks
    on_d = dp("ones1", [1, 128], F16, isOutput=False)
    out_d = dp("out", [S, DM], F16, isOutput=True)   # partial (pre-bo) projection
    scr_d = dp("scratch", [1, 8], F32, isOutput=True)  # keeps warm-up mms alive

    with tile.TileContext(nc) as tc:
        with tc.tile_pool(name="weights", bufs=1) as wpool, \
             tc.tile_pool(name="persist", bufs=1) as perst, \
             tc.tile_pool(name="xres", bufs=1) as xres:
            wq_sb = wpool.tile([128, MC, CD], F16)
            wk_sb = wpool.tile([128, MC, CD], F16)
            wv_sb = wpool.tile([128, MC, W260], F16)
            wo_sb = wpool.tile([128, 2, DM], F16)
            bq_sb = wpool.tile([128, 2], F32)
            bk_sb = wpool.tile([128, 2], F32)
            bv_sb = wpool.tile([1, W260], F16)
            on_sb = wpool.tile([1, 128], F16)

            xq_sb = xres.tile([128, MC, S], F16)
            xk_sb = xres.tile([128, MC, S], F16)
            xv_sb = xres.tile([128, MC, S], F16)

            qt_sb = perst.tile([128, 2, S], F16)   # [d(2 heads), pair, q]
            kt_sb = perst.tile([128, 2, S], F16)
            at_sb = perst.tile([128, 2, S], F16)   # normalized attn out^T
            v8hi = perst.tile([128, NPAIR, HLOC, 2, VP], F8)
            v8lo = perst.tile([128, NPAIR, HLOC, 2, VP], F8)

            # ---- prologue DMAs, split across both HWDGE queues (SP + ACT),
            # one contiguous 1MB blob per 512-col block, deadline order ----
            def dma_x(eng, x_sb, x_d, b):
                eng.dma_start(out=x_sb[:, :, b * 512:(b + 1) * 512],
                              in_=x_d.ap()[b * 128:(b + 1) * 128, :])

            # SP queue: Q-proj + K-proj critical path, first-needed-first
            nc.sync.dma_start(out=wq_sb[:], in_=wq_d.ap())
            nc.sync.dma_start(out=bq_sb[:], in_=bq_d.ap())
            nc.sync.dma_start(out=wk_sb[:], in_=wk_d.ap())
            nc.sync.dma_start(out=bk_sb[:], in_=bk_d.ap())
            for b in range(4):
                dma_x(nc.sync, xk_sb, xk_d, b)       # K proj jb0..3
            for b in range(1, 4):
                dma_x(nc.sync, xq_sb, xq_d, b)       # Q proj qq1..3

            # ACT queue: Q-proj qq0 input + V-proj critical path + wo
            nc.scalar.dma_start(out=xq_sb[:, 0:4, 0:512],
                                in_=xq_d.ap()[0:128, 0:2048])
            nc.scalar.dma_start(out=xq_sb[:, 4:8, 0:512],
                                in_=xq_d.ap()[0:128, 2048:4096])
            nc.scalar.dma_start(out=bv_sb[:], in_=bv_d.ap())
            nc.scalar.dma_start(out=on_sb[:], in_=on_d.ap())
            nc.scalar.dma_start(out=wv_sb[:], in_=wv_d.ap())
            for b in range(4):
                dma_x(nc.scalar, xv_sb, xv_d, b)     # V proj kt 4b..4b+3
            nc.scalar.dma_start(out=wo_sb[:], in_=wo_d.ap())

            with tc.tile_pool(name="psc", bufs=2, space="PSUM") as psc, \
                 tc.tile_pool(name="put", bufs=2, space="PSUM") as put, \
                 tc.tile_pool(name="paux", bufs=2, space="PSUM") as paux, \
                 tc.tile_pool(name="ex8p", bufs=4) as ex8p, \
                 tc.tile_pool(name="obuf", bufs=2) as obuf, \
                 tc.tile_pool(name="npool", bufs=2) as npool:

                open_tiles = {}

                def qproj(qq, p, step=None):
                    # step None: all 8 mm; else 2 mm per step (0..3), spread
                    # across entries to stay under the ACT pacing budget
                    steps = range(4) if step is None else (step,)
                    for s in steps:
                        if s == 0:
                            open_tiles["q"] = paux.tile([128, 512], F32,
                                                        tag="aux", name=f"q{qq}{p}")
                        t = open_tiles["q"]
                        for m in (2 * s, 2 * s + 1):
                            nc.tensor.matmul(
                                t[:], wq_sb[:, m, p * 128:(p + 1) * 128],
                                xq_sb[:, m, qq * 512:(qq + 1) * 512],
                                start=(m == 0), stop=(m == MC - 1))
                        if s == 3:
                            nc.vector.tensor_scalar_add(
                                qt_sb[:, p, qq * 512:(qq + 1) * 512],
                                t[:], bq_sb[:, p:p + 1])

                def kproj(jb, p):
                    t = paux.tile([128, 512], F32, tag="aux", name=f"k{jb}{p}")
                    for m in range(MC):
                        nc.tensor.matmul(
                            t[:], wk_sb[:, m, p * 128:(p + 1) * 128],
                            xk_sb[:, m, jb * 512:(jb + 1) * 512],
                            start=(m == 0), stop=(m == MC - 1))
                    nc.vector.tensor_scalar_add(
                        kt_sb[:, p, jb * 512:(jb + 1) * 512], t[:], bk_sb[:, p:p + 1])

                def vproj(kt):
                    t = paux.tile([128, 512], F32, tag="aux", name=f"v{kt}")
                    nc.tensor.matmul(t[:, 0:W260], on_sb[:], bv_sb[:],
                                     start=True, stop=False)
                    for m in range(MC):
                        nc.tensor.matmul(
                            t[:, 0:W260],
                            xv_sb[:, m, kt * 128:(kt + 1) * 128],
                            wv_sb[:, m, :],
                            start=False, stop=(m == MC - 1))
                    pr, pl = kt // 2, kt % 2
                    # hi = fp8(v'), lo = fp8(v' - hi); 4 heads in one strided op
                    nc.vector.tensor_copy(
                        v8hi[:, pr, :, pl, 0:VW], t[:, 0:W260])
                    nc.vector.tensor_tensor(
                        out=v8lo[:, pr, :, pl, 0:VW], in0=t[:, 0:W260],
                        in1=v8hi[:, pr, :, pl, 0:VW], op=SUB)

                def outproj(ot, half=None, act_copy=False):
                    # half 0: cc=0 matmuls (start); half 1: cc=1 (stop) +
                    # copies + store.  half None: both.  act_copy: offload
                    # one PSUM->SBUF copy to the (idle) scalar engine.
                    halves = range(2) if half is None else (half,)
                    for cc in halves:
                        if cc == 0:
                            open_tiles["o"] = [
                                paux.tile([128, 512], F32, tag="aux",
                                          name=f"o{ot}{n}") for n in range(2)]
                        os_ = open_tiles["o"]
                        for n in range(2):
                            nc.tensor.matmul(
                                os_[n][:],
                                at_sb[:, cc, ot * 128:(ot + 1) * 128],
                                wo_sb[:, cc, n * 512:(n + 1) * 512],
                                start=(cc == 0), stop=(cc == 1))
                        if cc == 1:
                            ob = obuf.tile([128, 1024], F16, tag="ob", name="ob")
                            nc.vector.tensor_copy(ob[:, 0:512], os_[0][:])
                            if act_copy:
                                nc.scalar.copy(ob[:, 512:1024], os_[1][:])
                            else:
                                nc.vector.tensor_copy(ob[:, 512:1024], os_[1][:])
                            nc.sync.dma_start(
                                out=out_d.ap()[ot * 128:(ot + 1) * 128, :],
                                in_=ob[:])

                def normalize(qq, p, uts):
                    rs = []
                    for hh in range(2):
                        den = npool.tile([1, 512], F32, tag="den", name=f"d{hh}")
                        nc.vector.tensor_scalar_add(den[:], uts[hh][64:65, :], 1.0)
                        r = npool.tile([1, 512], F32, tag="r", name=f"r{hh}")
                        nc.vector.reciprocal_approx_fast(r[:], den[:])
                        rs.append(r)
                    rbs = []
                    for hh in range(2):
                        rb = npool.tile([64, 512], F32, tag="rb", name=f"b{hh}")
                        nc.gpsimd.partition_broadcast(rb[:], rs[hh][:])
                        rbs.append(rb)
                    for hh in range(2):
                        nc.vector.tensor_tensor(
                            out=at_sb[64 * hh:64 * hh + 64, p, qq * 512:(qq + 1) * 512],
                            in0=uts[hh][0:64, :], in1=rbs[hh][:], op=MULT)

                # ---- prologue compute: bare minimum before entry (0,0,0);
                # everything else rides the crunch-quarter filler slots ----
                qproj(0, 0)
                kproj(0, 0)

                # ---- main pipeline ----
                def u_accum(uts, p, j, exj, hh):
                    h = 2 * p + hh
                    nc.tensor.matmul(
                        uts[hh][:], v8hi[:, j, h, :, 0:VW], exj[:, :, hh, :],
                        start=(j == 0), stop=False, perf_mode=DR)
                    nc.tensor.matmul(
                        uts[hh][:], v8lo[:, j, h, :, 0:VW], exj[:, :, hh, :],
                        start=False, stop=(j == NPAIR - 1), perf_mode=DR)

                carry = None   # (uts, p, qq, exj) of the previous quarter
                for qq in range(4):
                    for p in range(2):
                        uts = (put.tile([65, 512], F32, tag="ut", name="ut0"),
                               put.tile([65, 512], F32, tag="ut", name="ut1"))
                        exi = None
                        expairs = {}
                        for i in range(NKT):
                            j = i // 2
                            sc = psc.tile([128, 1024], F32, tag="sc", name="sc")
                            for hh in range(2):
                                nc.tensor.matmul(
                                    sc[:, hh * 512:(hh + 1) * 512],
                                    kt_sb[64 * hh:64 * hh + 64, p, i * 128:(i + 1) * 128],
                                    qt_sb[64 * hh:64 * hh + 64, p, qq * 512:(qq + 1) * 512],
                                    start=True, stop=True)
                            if i % 2 == 0:
                                exi = ex8p.tile([128, 2, 2, 512], F8, tag="ex",
                                                name="ex")
                                expairs[j] = exi
                            nc.scalar.activation(out=exi[:, i % 2, :, :],
                                                 in_=sc[:], func=EXP)
                            # ---- interleaved fillers ----
                            if qq == 0 and p == 0:
                                if i == 0:
                                    qproj(0, 1)
                                else:
                                    vproj(i - 1)
                                    if i == 15:
                                        vproj(15)
                                if i in (2, 6, 10):
                                    kproj(i // 4 + 1, 0)
                                if i in (4, 8, 12, 14):
                                    kproj((i - 4) // 4 if i < 14 else 3, 1)
                            else:
                                if p == 1 and qq < 3 and 1 <= i <= 4:
                                    qproj(qq + 1, 0, step=i - 1)
                                if p == 1 and qq < 3 and 7 <= i <= 10:
                                    qproj(qq + 1, 1, step=i - 7)
                                if p == 0 and qq > 0 and 7 <= i <= 14:
                                    ot = (qq - 1) * 4 + (i - 7) // 2
                                    outproj(ot, half=(i - 7) % 2)
                            # ---- previous quarter's last U halves + normalize,
                            # carried into entries 1-3 so they never
                            # head-of-line block this quarter's scores ----
                            if 1 <= i <= 3 and carry is not None:
                                cu, cp, cqq, cex = carry
                                if i == 1:
                                    u_accum(cu, cp, NPAIR - 2, cex[NPAIR - 2], hh=1)
                                elif i == 2:
                                    u_accum(cu, cp, NPAIR - 1, cex[NPAIR - 1], hh=0)
                                else:
                                    u_accum(cu, cp, NPAIR - 1, cex[NPAIR - 1], hh=1)
                                    normalize(cqq, cp, cu)
                                    carry = None
                            # ---- U accumulation, one head per entry, lag 2 ----
                            if i >= 3 and i % 2 == 1:
                                u_accum(uts, p, (i - 3) // 2,
                                        expairs[(i - 3) // 2], hh=0)
                            if i >= 4 and i % 2 == 0:
                                jd = (i - 4) // 2
                                u_accum(uts, p, jd, expairs[jd], hh=1)
                        carry = (uts, p, qq, expairs)
                        expairs = {}
                # ---- epilogue: final quarter's U + normalize + output tiles ----
                cu, cp, cqq, cex = carry
                u_accum(cu, cp, NPAIR - 2, cex[NPAIR - 2], hh=1)
                u_accum(cu, cp, NPAIR - 1, cex[NPAIR - 1], hh=0)
                u_accum(cu, cp, NPAIR - 1, cex[NPAIR - 1], hh=1)
                normalize(cqq, cp, cu)
                # keep the PE warm (HAM 8/8) while the normalize chain runs.
                # Each mm READS the final exp pair so the scheduler cannot
                # hoist them earlier; the scratch store keeps them unpruned.
                wps = paux.tile([128, 512], F32, tag="aux", name="warm")
                wex = cex[NPAIR - 1]
                for w in range(20):
                    nc.tensor.matmul(wps[:], wex[:, 0, 0, 0:128],
                                     wex[:, w % 2, 0, :],
                                     start=(w == 0), stop=(w == 19))
                scr = obuf.tile([1, 8], F32, tag="scr", name="scr")
                nc.vector.tensor_copy(scr[:], wps[0:1, 0:8])
                nc.sync.dma_start(out=scr_d.ap(), in_=scr[:])
                for ot in range(12, 16):
                    outproj(ot)

    nc.finalize()
    return nc


def kernel(query, key, value, Wq, bq, Wk, bk, Wv, bv, Wo, bo):
    global LAST_RESULT
    if "nc" not in _CACHE:
        _CACHE["nc"] = _build()
    nc = _CACHE["nc"]

    query = np.asarray(query, np.float32)
    key = np.asarray(key, np.float32)
    value = np.asarray(value, np.float32)
    Wq = np.asarray(Wq, np.float32)
    Wk = np.asarray(Wk, np.float32)
    Wv = np.asarray(Wv, np.float32)
    Wo = np.asarray(Wo, np.float32)
    bq = np.asarray(bq, np.float32)
    bk = np.asarray(bk, np.float32)
    bv = np.asarray(bv, np.float32)
    bo = np.asarray(bo, np.float32)

    def blob(x):
        # [S, DM] -> x^T [DM, S] -> [4*128, MC*512]: block b rows hold
        # [d 128][m 8][512] contiguously for single-descriptor-chain DMAs
        xT = x.T.astype(np.float16)                       # [DM, S]
        t = xT.reshape(MC, 128, 4, 512).transpose(2, 1, 0, 3)
        return np.ascontiguousarray(t.reshape(4 * 128, MC * 512))

    xqT = [blob(query[b]) for b in range(B)]
    xkT = [blob(key[b]) for b in range(B)]
    xvT = [blob(value[b]) for b in range(B)]

    def wblob(wT, nch):
        # [nch*128, C] -> [128, nch*C] matching SBUF [128][chunk][C] tiles
        C = wT.shape[1]
        t = wT.reshape(nch, 128, C).transpose(1, 0, 2)
        return np.ascontiguousarray(t.reshape(128, nch * C)).astype(np.float16)

    ones1 = np.ones((1, 128), np.float16)
    in_maps = []
    for c in range(8):
        b, hg = c // 4, c % 4
        r0 = hg * CD
        wq_s = wblob((Wq[r0:r0 + CD, :] * SCALE).T, MC)
        wk_s = wblob(Wk[r0:r0 + CD, :].T, MC)
        wo_s = wblob(Wo[:, r0:r0 + CD].T, 2)
        bq_s = np.ascontiguousarray((bq[r0:r0 + CD] * SCALE).reshape(2, 128).T)
        bk_s = np.ascontiguousarray(bk[r0:r0 + CD].reshape(2, 128).T)
        # V weights/bias in 260-layout: [64 cols of head | bias-1 col] x4
        wv260 = np.zeros((DM, W260), np.float32)
        bv260 = np.zeros((1, W260), np.float32)
        for hh in range(HLOC):
            wv260[:, hh * VW:hh * VW + HD] = Wv[r0 + hh * HD:r0 + (hh + 1) * HD, :].T
            bv260[0, hh * VW:hh * VW + HD] = bv[r0 + hh * HD:r0 + (hh + 1) * HD]
            bv260[0, hh * VW + HD] = 1.0
        in_maps.append({
            "xq": xqT[b], "xk": xkT[b], "xv": xvT[b],
            "wq": wq_s, "wk": wk_s, "wv": wblob(wv260, MC),
            "wo": wo_s, "bq": bq_s, "bk": bk_s, "bv": bv260.astype(np.float16),
            "ones1": ones1,
        })

    res = run_bass_kernel_spmd(nc, in_maps, core_ids=list(range(8)))
    LAST_RESULT = res

    out = np.empty((B, S, DM), np.float32)
    for b in range(B):
        acc = np.zeros((S, DM), np.float64)
        for hg in range(4):
            acc += res.results[b * 4 + hg]["out"]
        out[b] = (acc + bo.astype(np.float64)).astype(np.float32)
    return out


# revision 60
# speedup vs baseline: 1.2490x; 1.0100x over previous
"""Multi-head attention (softmax+1) for TRN2, 8 NeuronCores.  ~228us.

Sharding: data-parallel over batch B=2 (4 cores per batch) x tensor-parallel
over the 16 heads (4 heads per core).  Each core computes its 4 heads'
QKV projections, attention, and a partial output projection; the host sums
the 4 f16 partials per batch and adds the output bias.

Single software-pipelined loop over 128 (qq, p, ktile) entries:
  scores^T[k,q] (f16 PE matmuls, head-paired row groups, K=64) -> exp on the
  scalar engine written DIRECTLY AS fp8e4 into DoubleRow k-plane pairs
  [128,2,2head,512] -> U^T accumulated with fp8 DoubleRow matmuls (0.5
  cyc/row; V' split into fp8 hi+lo on DVE so V keeps ~f16 precision; the
  ones-column gives the softmax+1 denominator in U row 64).  Measured
  rel err 1.74e-2 (vs 2e-2 gate), dominated by e4m3 rounding of exp —
  numpy-sim-predicted to 0.4% accuracy.

Scheduling (from perfetto/ntff traces):
 - U matmuls lag their exp pair by 2 entries (PE runs ~2 entries ahead on
   the psc double buffer; shorter lag head-of-line blocks scores on ACT).
 - The last U halves + normalize of each quarter carry into entries 1-3 of
   the next quarter; quarter boundaries stay gap-free.
 - K/V projections ride the first quarter (PE-bound "crunch", ~35us);
   Q projections ride p1 quarters (2 mm/entry), out-projection rides p0
   quarters (2 mm/entry, i>=7 so the carried normalize has slack).
 - x/w inputs are host-pre-tiled into contiguous blobs so each deadline-
   ordered DMA is one descriptor chain; split across both HWDGE queues
   (SP: wq/wk/xk/xq-bulk + out tiles; ACT-pre-exp: xq0/wv/xv/wo).
 - Epilogue: PE kept warm through the final normalize by fp8 dummy matmuls
   pinned (data-dependent) to the last exp; Tile schedules by dependency,
   not emission order, so dependency-free fillers get hoisted.
Engines at steady state: PE ~1.25us/entry (the wall), ACT 1.11, DVE ~0.6.
"""

import sys

if "/opt/trn_rl_repo" not in sys.path:
    sys.path.insert(0, "/opt/trn_rl_repo")

import numpy as np

import concourse.bass as bass
import concourse.mybir as mybir
import concourse.tile as tile
from concourse import bacc
from concourse.bass_utils import run_bass_kernel_spmd

F32 = mybir.dt.float32
F16 = mybir.dt.float16
F8 = mybir.dt.float8e4
EXP = mybir.ActivationFunctionType.Exp
DR = mybir.MatmulPerfMode.DoubleRow
SUB = mybir.AluOpType.subtract
MULT = mybir.AluOpType.mult

B, S, DM = 2, 2048, 1024
H, HD = 16, 64
SCALE = HD ** -0.5
HLOC = 4              # heads per core
CD = HLOC * HD        # 256 local head dims
VW = HD + 1           # 65: V columns + ones column per head
W260 = HLOC * VW      # 260
MC = DM // 128        # 8 contraction chunks for projections
NKT = S // 128        # 16 k tiles
NPAIR = NKT // 2      # 8 ktile pairs (fp8 DoubleRow planes)
VP = 80               # fp8 V' plane stride (pad 65 -> 80, 16-aligned)

_CACHE = {}
LAST_RESULT = None


def _build():
    nc = bacc.Bacc()
    dp = nc.declare_dram_parameter
    # x inputs pre-tiled on host into contiguous 512-col blocks:
    # blob[b][d 128][m 8][512] so each block DMA reads one contiguous 1MB
    xq_d = dp("xq", [4 * 128, MC * 512], F16, isOutput=False)
    xk_d = dp("xk", [4 * 128, MC * 512], F16, isOutput=False)
    xv_d = dp("xv", [4 * 128, MC * 512], F16, isOutput=False)
    # weights pre-shuffled on host to [128][chunk][cols] blob order so each
    # loads with a single dma_start
    wq_d = dp("wq", [128, MC * CD], F16, isOutput=False)   # (SCALE*Wq)^T blob
    wk_d = dp("wk", [128, MC * CD], F16, isOutput=False)
    wv_d = dp("wv", [128, MC * W260], F16, isOutput=False)
    wo_d = dp("wo", [128, 2 * DM], F16, isOutput=False)
    bq_d = dp("bq", [128, 2], F32, isOutput=False)   # bias cols per 128-pair
    bk_d = dp("bk", [128, 2], F32, isOutput=False)
    bv_d = dp("bv", [1, W260], F16, isOutput=False)  # [bv_h | 1.0] blocks
    on_d = dp("ones1", [1, 128], F16, isOutput=False)
    out_d = dp("out", [S, DM], F16, isOutput=True)   # partial (pre-bo) projection
    scr_d = dp("scratch", [1, 8], F32, isOutput=True)  # keeps warm-up mms alive

    with tile.TileContext(nc) as tc:
        with tc.tile_pool(name="weights", bufs=1) as wpool, \
             tc.tile_pool(name="persist", bufs=1) as perst, \
             tc.tile_pool(name="xres", bufs=1) as xres:
            wq_sb = wpool.tile([128, MC, CD], F16)
            wk_sb = wpool.tile([128, MC, CD], F16)
            wv_sb = wpool.tile([128, MC, W260], F16)
            wo_sb = wpool.tile([128, 2, DM], F16)
            bq_sb = wpool.tile([128, 2], F32)
            bk_sb = wpool.tile([128, 2], F32)
            bv_sb = wpool.tile([1, W260], F16)
            on_sb = wpool.tile([1, 128], F16)

            xq_sb = xres.tile([128, MC, S], F16)
            xk_sb = xres.tile([128, MC, S], F16)
            xv_sb = xres.tile([128, MC, S], F16)

            qt_sb = perst.tile([128, 2, S], F16)   # [d(2 heads), pair, q]
            kt_sb = perst.tile([128, 2, S], F16)
            at_sb = perst.tile([128, 2, S], F16)   # normalized attn out^T
            v8hi = perst.tile([128, NPAIR, HLOC, 2, VP], F8)
            v8lo = perst.tile([128, NPAIR, HLOC, 2, VP], F8)

            # ---- prologue DMAs, split across both HWDGE queues (SP + ACT),
            # one contiguous 1MB blob per 512-col block, deadline order ----
            def dma_x(eng, x_sb, x_d, b):
                eng.dma_start(out=x_sb[:, :, b * 512:(b + 1) * 512],
                              in_=x_d.ap()[b * 128:(b + 1) * 128, :])

            # SP queue: Q-proj + K-proj critical path, first-needed-first
            nc.sync.dma_start(out=wq_sb[:], in_=wq_d.ap())
            nc.sync.dma_start(out=bq_sb[:], in_=bq_d.ap())
            nc.sync.dma_start(out=wk_sb[:], in_=wk_d.ap())
            nc.sync.dma_start(out=bk_sb[:], in_=bk_d.ap())
            for b in range(4):
                dma_x(nc.sync, xk_sb, xk_d, b)       # K proj jb0..3
            for b in range(1, 4):
                dma_x(nc.sync, xq_sb, xq_d, b)       # Q proj qq1..3

            # ACT queue: Q-proj qq0 input + V-proj critical path + wo
            nc.scalar.dma_start(out=xq_sb[:, 0:4, 0:512],
                                in_=xq_d.ap()[0:128, 0:2048])
            nc.scalar.dma_start(out=xq_sb[:, 4:8, 0:512],
                                in_=xq_d.ap()[0:128, 2048:4096])
            nc.scalar.dma_start(out=bv_sb[:], in_=bv_d.ap())
            nc.scalar.dma_start(out=on_sb[:], in_=on_d.ap())
            nc.scalar.dma_start(out=wv_sb[:], in_=wv_d.ap())
            for b in range(4):
                dma_x(nc.scalar, xv_sb, xv_d, b)     # V proj kt 4b..4b+3
            nc.scalar.dma_start(out=wo_sb[:], in_=wo_d.ap())

            with tc.tile_pool(name="psc", bufs=2, space="PSUM") as psc, \
                 tc.tile_pool(name="put", bufs=2, space="PSUM") as put, \
                 tc.tile_pool(name="paux", bufs=2, space="PSUM") as paux, \
                 tc.tile_pool(name="ex8p", bufs=4) as ex8p, \
                 tc.tile_pool(name="obuf", bufs=2) as obuf, \
                 tc.tile_pool(name="npool", bufs=2) as npool:

                open_tiles = {}

                def qproj(qq, p, step=None):
                    # step None: all 8 mm; else 2 mm per step (0..3), spread
                    # across entries to stay under the ACT pacing budget
                    steps = range(4) if step is None else (step,)
                    for s in steps:
                        if s == 0:
                            open_tiles["q"] = paux.tile([128, 512], F32,
                                                        tag="aux", name=f"q{qq}{p}")
                        t = open_tiles["q"]
                        for m in (2 * s, 2 * s + 1):
                            nc.tensor.matmul(
                                t[:], wq_sb[:, m, p * 128:(p + 1) * 128],
                                xq_sb[:, m, qq * 512:(qq + 1) * 512],
                                start=(m == 0), stop=(m == MC - 1))
                        if s == 3:
                            nc.vector.tensor_scalar_add(
                                qt_sb[:, p, qq * 512:(qq + 1) * 512],
                                t[:], bq_sb[:, p:p + 1])

                def kproj(jb, p):
                    t = paux.tile([128, 512], F32, tag="aux", name=f"k{jb}{p}")
                    for m in range(MC):
                        nc.tensor.matmul(
                            t[:], wk_sb[:, m, p * 128:(p + 1) * 128],
                            xk_sb[:, m, jb * 512:(jb + 1) * 512],
                            start=(m == 0), stop=(m == MC - 1))
                    nc.vector.tensor_scalar_add(
                        kt_sb[:, p, jb * 512:(jb + 1) * 512], t[:], bk_sb[:, p:p + 1])

                def vproj(kt):
                    t = paux.tile([128, 512], F32, tag="aux", name=f"v{kt}")
                    nc.tensor.matmul(t[:, 0:W260], on_sb[:], bv_sb[:],
                                     start=True, stop=False)
                    for m in range(MC):
                        nc.tensor.matmul(
                            t[:, 0:W260],
                            xv_sb[:, m, kt * 128:(kt + 1) * 128],
                            wv_sb[:, m, :],
                            start=False, stop=(m == MC - 1))
                    pr, pl = kt // 2, kt % 2
                    # hi = fp8(v'), lo = fp8(v' - hi); 4 heads in one strided op
                    nc.vector.tensor_copy(
                        v8hi[:, pr, :, pl, 0:VW], t[:, 0:W260])
                    nc.vector.tensor_tensor(
                        out=v8lo[:, pr, :, pl, 0:VW], in0=t[:, 0:W260],
                        in1=v8hi[:, pr, :, pl, 0:VW], op=SUB)

                def outproj(ot, half=None, act_copy=False):
                    # half 0: cc=0 matmuls (start); half 1: cc=1 (stop) +
                    # copies + store.  half None: both.  act_copy: offload
                    # one PSUM->SBUF copy to the (idle) scalar engine.
                    halves = range(2) if half is None else (half,)
                    for cc in halves:
                        if cc == 0:
                            open_tiles["o"] = [
                                paux.tile([128, 512], F32, tag="aux",
                                          name=f"o{ot}{n}") for n in range(2)]
                        os_ = open_tiles["o"]
                        for n in range(2):
                            nc.tensor.matmul(
                                os_[n][:],
                                at_sb[:, cc, ot * 128:(ot + 1) * 128],
                                wo_sb[:, cc, n * 512:(n + 1) * 512],
                                start=(cc == 0), stop=(cc == 1))
                        if cc == 1:
                            ob = obuf.tile([128, 1024], F16, tag="ob", name="ob")
                            nc.vector.tensor_copy(ob[:, 0:512], os_[0][:])
                            if act_copy:
                                nc.scalar.copy(ob[:, 512:1024], os_[1][:])
                            else:
                                nc.vector.tensor_copy(ob[:, 512:1024], os_[1][:])
                            nc.sync.dma_start(
                                out=out_d.ap()[ot * 128:(ot + 1) * 128, :],
                                in_=ob[:])

                def normalize(qq, p, uts):
                    rs = []
                    for hh in range(2):
                        den = npool.tile([1, 512], F32, tag="den", name=f"d{hh}")
                        nc.vector.tensor_scalar_add(den[:], uts[hh][64:65, :], 1.0)
                        r = npool.tile([1, 512], F32, tag="r", name=f"r{hh}")
                        nc.vector.reciprocal_approx_fast(r[:], den[:])
                        rs.append(r)
                    rbs = []
                    for hh in range(2):
                        rb = npool.tile([64, 512], F32, tag="rb", name=f"b{hh}")
                        nc.gpsimd.partition_broadcast(rb[:], rs[hh][:])
                        rbs.append(rb)
                    for hh in range(2):
                        nc.vector.tensor_tensor(
                            out=at_sb[64 * hh:64 * hh + 64, p, qq * 512:(qq + 1) * 512],
                            in0=uts[hh][0:64, :], in1=rbs[hh][:], op=MULT)

                # ---- prologue compute: bare minimum before entry (0,0,0);
                # everything else rides the crunch-quarter filler slots ----
                qproj(0, 0)
                kproj(0, 0)

                # ---- main pipeline ----
                def u_accum(uts, p, j, exj, hh):
                    h = 2 * p + hh
                    nc.tensor.matmul(
                        uts[hh][:], v8hi[:, j, h, :, 0:VW], exj[:, :, hh, :],
                        start=(j == 0), stop=False, perf_mode=DR)
                    nc.tensor.matmul(
                        uts[hh][:], v8lo[:, j, h, :, 0:VW], exj[:, :, hh, :],
                        start=False, stop=(j == NPAIR - 1), perf_mode=DR)

                carry = None   # (uts, p, qq, exj) of the previous quarter
                for qq in range(4):
                    for p in range(2):
                        uts = (put.tile([65, 512], F32, tag="ut", name="ut0"),
                               put.tile([65, 512], F32, tag="ut", name="ut1"))
                        exi = None
                        expairs = {}
                        for i in range(NKT):
                            j = i // 2
                            sc = psc.tile([128, 1024], F32, tag="sc", name="sc")
                            for hh in range(2):
                                nc.tensor.matmul(
                                    sc[:, hh * 512:(hh + 1) * 512],
                                    kt_sb[64 * hh:64 * hh + 64, p, i * 128:(i + 1) * 128],
                                    qt_sb[64 * hh:64 * hh + 64, p, qq * 512:(qq + 1) * 512],
                                    start=True, stop=True)
                            if i % 2 == 0:
                                exi = ex8p.tile([128, 2, 2, 512], F8, tag="ex",
                                                name="ex")
                                expairs[j] = exi
                            nc.scalar.activation(out=exi[:, i % 2, :, :],
                                                 in_=sc[:], func=EXP)
                            # ---- interleaved fillers ----
                            if qq == 0 and p == 0:
                                if i == 0:
                                    qproj(0, 1)
                                else:
                                    vproj(i - 1)
                                    if i == 15:
                                        vproj(15)
                                if i in (2, 6, 10):
                                    kproj(i // 4 + 1, 0)
                                if i in (4, 8, 12, 14):
                                    kproj((i - 4) // 4 if i < 14 else 3, 1)
                            else:
                                if p == 1 and qq < 3 and 1 <= i <= 4:
                                    qproj(qq + 1, 0, step=i - 1)
                                if p == 1 and qq < 3 and 7 <= i <= 10:
                                    qproj(qq + 1, 1, step=i - 7)
                                if p == 0 and qq > 0 and 7 <= i <= 14:
                                    ot = (qq - 1) * 4 + (i - 7) // 2
                                    outproj(ot, half=(i - 7) % 2)
                            # ---- previous quarter's last U halves + normalize,
                            # carried into entries 1-3 so they never
                            # head-of-line block this quarter's scores ----
                            if 1 <= i <= 3 and carry is not None:
                                cu, cp, cqq, cex = carry
                                if i == 1:
                                    u_accum(cu, cp, NPAIR - 2, cex[NPAIR - 2], hh=1)
                                elif i == 2:
                                    u_accum(cu, cp, NPAIR - 1, cex[NPAIR - 1], hh=0)
                                else:
                                    u_accum(cu, cp, NPAIR - 1, cex[NPAIR - 1], hh=1)
                                    normalize(cqq, cp, cu)
                                    carry = None
                            # ---- U accumulation, one head per entry, lag 2 ----
                            if i >= 3 and i % 2 == 1:
                                u_accum(uts, p, (i - 3) // 2,
                                        expairs[(i - 3) // 2], hh=0)
                            if i >= 4 and i % 2 == 0:
                                jd = (i - 4) // 2
                                u_accum(uts, p, jd, expairs[jd], hh=1)
                        carry = (uts, p, qq, expairs)
                        expairs = {}
                # ---- epilogue: final quarter's U + normalize + output tiles ----
                cu, cp, cqq, cex = carry
                u_accum(cu, cp, NPAIR - 2, cex[NPAIR - 2], hh=1)
                u_accum(cu, cp, NPAIR - 1, cex[NPAIR - 1], hh=0)
                u_accum(cu, cp, NPAIR - 1, cex[NPAIR - 1], hh=1)
                normalize(cqq, cp, cu)
                # keep the PE warm (HAM 8/8) while the normalize chain runs.
                # Each mm READS the final exp pair so the scheduler cannot
                # hoist them earlier; the scratch store keeps them unpruned.
                wps = paux.tile([128, 512], F32, tag="aux", name="warm")
                wex = cex[NPAIR - 1]
                for w in range(20):
                    nc.tensor.matmul(wps[:], wex[:, 0, 0, 0:128],
                                     wex[:, w % 2, 0, :],
                                     start=(w == 0), stop=(w == 19))
                scr = obuf.tile([1, 8], F32, tag="scr", name="scr")
                nc.vector.tensor_copy(scr[:], wps[0:1, 0:8])
                nc.sync.dma_start(out=scr_d.ap(), in_=scr[:])
                for ot in range(12, 16):
                    outproj(ot)

    nc.finalize()
    return nc


def kernel(query, key, value, Wq, bq, Wk, bk, Wv, bv, Wo, bo):
    global LAST_RESULT
    if "nc" not in _CACHE:
        _CACHE["nc"] = _build()
    nc = _CACHE["nc"]

    query = np.asarray(query, np.float32)
    key = np.asarray(key, np.float32)
    value = np.asarray(value, np.float32)
    Wq = np.asarray(Wq, np.float32)
    Wk = np.asarray(Wk, np.float32)
    Wv = np.asarray(Wv, np.float32)
    Wo = np.asarray(Wo, np.float32)
    bq = np.asarray(bq, np.float32)
    bk = np.asarray(bk, np.float32)
    bv = np.asarray(bv, np.float32)
    bo = np.asarray(bo, np.float32)

    def blob(x):
        # [S, DM] -> x^T [DM, S] -> [4*128, MC*512]: block b rows hold
        # [d 128][m 8][512] contiguously for single-descriptor-chain DMAs
        xT = x.T.astype(np.float16)                       # [DM, S]
        t = xT.reshape(MC, 128, 4, 512).transpose(2, 1, 0, 3)
        return np.ascontiguousarray(t.reshape(4 * 128, MC * 512))

    xqT = [blob(query[b]) for b in range(B)]
    xkT = [blob(key[b]) for b in range(B)]
    xvT = [blob(value[b]) for b in range(B)]

    def wblob(wT, nch):
        # [nch*128, C] -> [128, nch*C] matching SBUF [128][chunk][C] tiles
        C = wT.shape[1]
        t = wT.reshape(nch, 128, C).transpose(1, 0, 2)
        return np.ascontiguousarray(t.reshape(128, nch * C)).astype(np.float16)

    ones1 = np.ones((1, 128), np.float16)
    in_maps = []
    for c in range(8):
        b, hg = c // 4, c % 4
        r0 = hg * CD
        wq_s = wblob((Wq[r0:r0 + CD, :] * SCALE).T, MC)
        wk_s = wblob(Wk[r0:r0 + CD, :].T, MC)
        wo_s = wblob(Wo[:, r0:r0 + CD].T, 2)
        bq_s = np.ascontiguousarray((bq[r0:r0 + CD] * SCALE).reshape(2, 128).T)
        bk_s = np.ascontiguousarray(bk[r0:r0 + CD].reshape(2, 128).T)
        # V weights/bias in 260-layout: [64 cols of head | bias-1 col] x4
        wv260 = np.zeros((DM, W260), np.float32)
        bv260 = np.zeros((1, W260), np.float32)
        for hh in range(HLOC):
            wv260[:, hh * VW:hh * VW + HD] = Wv[r0 + hh * HD:r0 + (hh + 1) * HD, :].T
            bv260[0, hh * VW:hh * VW + HD] = bv[r0 + hh * HD:r0 + (hh + 1) * HD]
            bv260[0, hh * VW + HD] = 1.0
        in_maps.append({
            "xq": xqT[b], "xk": xkT[b], "xv": xvT[b],
            "wq": wq_s, "wk": wk_s, "wv": wblob(wv260, MC),
            "wo": wo_s, "bq": bq_s, "bk": bk_s, "bv": bv260.astype(np.float16),
            "ones1": ones1,
        })

    res = run_bass_kernel_spmd(nc, in_maps, core_ids=list(range(8)))
    LAST_RESULT = res

    out = np.empty((B, S, DM), np.float32)
    for b in range(B):
        acc = np.zeros((S, DM), np.float64)
        for hg in range(4):
            acc += res.results[b * 4 + hg]["out"]
        out[b] = (acc + bo.astype(np.float64)).astype(np.float32)
    return out
